# revision 7
# baseline (speedup 1.0000x reference)
"""Trainium2 Bass kernel: location-sensitive attention (Tacotron2-style).

Contract: kernel(**inputs) takes the FULL numpy inputs and returns the full
(attention_context [B, EMB], attention_weights [B, T]) tuple.

Sharding: data-parallel over batch across 8 NeuronCores (8 batches/core),
weights replicated.  All tensor math runs on-device; host work is limited to
zero-flop layout prep (slicing, padding, transposes / constant folding of the
tiny weights).
"""

import contextlib
import ctypes
import sys
import types

import numpy as np

import concourse.bacc as bacc
import concourse.bass as bass
import concourse.tile as tile
from concourse import mybir
from concourse.bass_utils import run_bass_kernel_spmd

F32 = mybir.dt.float32

B, T = 64, 2048
RNN, EMB, ATT, NF, KW = 1024, 512, 128, 32, 31
PAD = (KW - 1) // 2
NCORES = 8
BPC = B // NCORES          # batches per core
TP = T + 2 * PAD           # padded conv length
KC = 2 * KW                # im2col contraction dim (c, k)
NCHUNK = T // 128          # 16 t-chunks of 128


def _install_ntff_shim():
    """Provide antenv.axon_hooks (absent in some images) so trace=True can
    capture NTFF profiles through libaxon_pjrt.so.  No-op if unavailable."""
    if "antenv.axon_hooks" in sys.modules:
        return
    hook = None
    try:
        lib = ctypes.CDLL("/opt/axon/libaxon_pjrt.so")
        if hasattr(lib, "axon_start_nrt_profile"):
            lib.axon_start_nrt_profile.argtypes = [
                ctypes.POINTER(ctypes.c_int64),
                ctypes.c_size_t,
            ]
            lib.axon_start_nrt_profile.restype = ctypes.c_int64
            lib.axon_stop_nrt_profile.argtypes = [ctypes.c_char_p]
            lib.axon_stop_nrt_profile.restype = ctypes.c_int64

            @contextlib.contextmanager
            def _hook(output_dir, device_ids):
                import jax

                jax.devices()
                if device_ids:
                    ids = (ctypes.c_int64 * len(device_ids))(*device_ids)
                    rc = lib.axon_start_nrt_profile(ids, len(device_ids))
                else:
                    rc = lib.axon_start_nrt_profile(None, 0)
                if rc != 0:
                    raise RuntimeError(f"axon_start_nrt_profile rc={rc}")
                try:
                    yield
                finally:
                    n = lib.axon_stop_nrt_profile(str(output_dir).encode())
                    print(f"ntff profile: {n} file(s) in {output_dir}", file=sys.stderr)

            hook = _hook
    except OSError:
        pass
    mod = types.ModuleType("antenv.axon_hooks")
    mod.get_axon_ntff_profile_hook = lambda: hook
    mod.set_axon_ntff_profile_hook = lambda h: None
    sys.modules["antenv.axon_hooks"] = mod


def build_program():
    """Build and compile the per-core Bass program (SPMD, identical on all
    cores; only the input data differs)."""
    nc = bacc.Bacc("TRN2", target_bir_lowering=False, debug=False,
                   num_devices=NCORES)

    # ---- DRAM tensors (per-core shapes) ----
    d_ahsT = nc.dram_tensor("ahsT", [RNN, BPC], F32, kind="ExternalInput")
    d_wqT = nc.dram_tensor("wqT", [RNN, ATT], F32, kind="ExternalInput")
    d_blhs = nc.dram_tensor("blhs", [NF + 2, BPC], F32, kind="ExternalInput")
    d_brhs = nc.dram_tensor("brhs", [NF + 2, ATT], F32, kind="ExternalInput")
    d_awc = nc.dram_tensor("awc", [BPC, 2, TP], F32, kind="ExternalInput")
    d_wcomb = nc.dram_tensor("wcomb", [KC, ATT], F32, kind="ExternalInput")
    d_ones = nc.dram_tensor("ones", [T], F32, kind="ExternalInput")
    d_v = nc.dram_tensor("vrow", [ATT], F32, kind="ExternalInput")
    d_ident = nc.dram_tensor("ident", [128, 128], F32, kind="ExternalInput")
    d_pm = nc.dram_tensor("pm", [BPC, T, ATT], F32, kind="ExternalInput")
    d_mem = nc.dram_tensor("mem", [BPC, T, EMB], F32, kind="ExternalInput")
    d_mask = nc.dram_tensor("maskb", [BPC, T], mybir.dt.uint8,
                            kind="ExternalInput")
    d_octx = nc.dram_tensor("out_ctx", [BPC, EMB], F32, kind="ExternalOutput")
    d_ow = nc.dram_tensor("out_w", [BPC, T], F32, kind="ExternalOutput")

    with tile.TileContext(nc) as tc, contextlib.ExitStack() as ctx:
        consts = ctx.enter_context(tc.tile_pool(name="consts", bufs=1))
        xpool = ctx.enter_context(tc.tile_pool(name="xpool", bufs=3))
        rhspool = ctx.enter_context(tc.tile_pool(name="rhspool", bufs=3))
        pmpool = ctx.enter_context(tc.tile_pool(name="pmpool", bufs=2))
        mempool = ctx.enter_context(tc.tile_pool(name="mempool", bufs=3))
        ypool = ctx.enter_context(tc.tile_pool(name="ypool", bufs=3))
        small = ctx.enter_context(tc.tile_pool(name="small", bufs=3))
        p_e = ctx.enter_context(tc.tile_pool(name="p_e", bufs=3, space="PSUM"))
        p_ctx = ctx.enter_context(tc.tile_pool(name="p_ctx", bufs=2, space="PSUM"))
        p_misc = ctx.enter_context(tc.tile_pool(name="p_misc", bufs=3, space="PSUM"))

        # ---- resident constants ----
        wqT_sb = consts.tile([128, RNN // 128, ATT], F32)
        nc.sync.dma_start(out=wqT_sb, in_=d_wqT.ap().rearrange(
            "(a p) n -> p a n", p=128))
        ahsT_sb = consts.tile([128, RNN // 128, BPC], F32)
        nc.sync.dma_start(out=ahsT_sb, in_=d_ahsT.ap().rearrange(
            "(a p) b -> p a b", p=128))
        blhs_sb = consts.tile([NF + 2, BPC], F32)
        nc.sync.dma_start(out=blhs_sb, in_=d_blhs.ap())
        brhs_sb = consts.tile([NF + 2, ATT], F32)
        nc.sync.dma_start(out=brhs_sb, in_=d_brhs.ap())
        v_sb = consts.tile([128, ATT], F32)
        nc.sync.dma_start(out=v_sb, in_=bass.AP(
            tensor=d_v, offset=0, ap=[[0, 128], [1, ATT]]))
        ident_sb = consts.tile([128, 128], F32)
        nc.sync.dma_start(out=ident_sb, in_=d_ident.ap())
        ones_col = consts.tile([128, 1], F32)
        nc.sync.dma_start(out=ones_col, in_=d_ones.ap()[:128].rearrange(
            "(p o) -> p o", o=1))
        ones_row = consts.tile([1, 128], F32)
        nc.sync.dma_start(out=ones_row, in_=d_ones.ap()[:128].rearrange(
            "(o q) -> o q", o=1))
        pq_sb = consts.tile([BPC, ATT], F32)

        # ---- processed query + all bias terms, one accumulation group ----
        # pq[b, a] = sum_r ahs[b, r] Wq[a, r] + bq[a] + bl[a] + (conv_b @ Wl.T)[a]
        ppq = p_misc.tile([BPC, ATT], F32, tag="pmsc")
        for a in range(RNN // 128):
            nc.tensor.matmul(ppq, lhsT=ahsT_sb[:, a, :], rhs=wqT_sb[:, a, :],
                             start=(a == 0), stop=False)
        nc.tensor.matmul(ppq, lhsT=blhs_sb, rhs=brhs_sb, start=False, stop=True)
        nc.scalar.copy(pq_sb, ppq)

        for b in range(BPC):
            # ---- X im2col [63, T]: rows (c,k) = awc_pad[b, c, k:k+T], row 62 = ones
            # (PE stationary operand must sit at base partition 0/32/64)
            xt = xpool.tile([KC + 1, T], F32)
            nc.sync.dma_start(
                out=xt[:KC, :],
                in_=bass.AP(tensor=d_awc, offset=b * 2 * TP,
                            ap=[[TP, 2], [1, KW], [1, T]]))
            nc.sync.dma_start(
                out=xt[KC:KC + 1, :],
                in_=d_ones.ap().rearrange("(o t) -> o t", o=1))

            # rhs for the fused conv+projection+bias matmul: [Wcomb; pq[b]]
            rhs_b = rhspool.tile([KC + 1, ATT], F32)
            nc.gpsimd.dma_start(out=rhs_b[:KC, :], in_=d_wcomb.ap())
            nc.gpsimd.dma_start(out=rhs_b[KC:KC + 1, :], in_=pq_sb[b:b + 1, :])

            pm_t = pmpool.tile([128, NCHUNK, ATT], F32)
            nc.sync.dma_start(out=pm_t, in_=d_pm.ap()[b].rearrange(
                "(c p) a -> p c a", p=128))

            e_b = small.tile([128, NCHUNK], F32, tag="e_b")
            for c in range(NCHUNK):
                pe = p_e.tile([128, ATT], F32)
                nc.tensor.matmul(
                    pe, lhsT=xt[:, bass.ts(c, 128)],
                    rhs=rhs_b, start=True, stop=True)
                y = ypool.tile([128, ATT], F32, tag="y")
                nc.vector.tensor_add(y, pe, pm_t[:, c, :])
                th = ypool.tile([128, ATT], F32, tag="th")
                nc.scalar.activation(th, y, mybir.ActivationFunctionType.Tanh)
                prod = ypool.tile([128, ATT], F32, tag="prod")
                nc.vector.tensor_mul(prod, th, v_sb)
                # (+v_b dropped: softmax is shift-invariant, so it cancels)
                nc.vector.tensor_reduce(
                    out=e_b[:, c:c + 1], in_=prod,
                    axis=mybir.AxisListType.X, op=mybir.AluOpType.add)

            # ---- mask: energies += -1e30 * mask ----
            m8 = small.tile([128, NCHUNK], mybir.dt.uint8, tag="m8")
            nc.gpsimd.dma_start(out=m8, in_=d_mask.ap()[b].rearrange(
                "(c p) -> p c", p=128))
            mf = small.tile([128, NCHUNK], F32, tag="mf")
            nc.vector.tensor_copy(mf, m8)
            nc.vector.tensor_scalar_mul(mf, mf, -1e30)
            nc.vector.tensor_add(e_b, e_b, mf)

            # ---- softmax over T (energies bounded by sum|v|: no max pass) ----
            w_b = small.tile([128, NCHUNK], F32, tag="w_b")
            s_col = small.tile([128, 1], F32, tag="s_col")
            nc.scalar.activation(w_b, e_b, mybir.ActivationFunctionType.Exp,
                                 accum_out=s_col)
            p_tot = p_misc.tile([1, 1], F32, tag="pmsc")
            nc.tensor.matmul(p_tot, lhsT=s_col, rhs=ones_col,
                             start=True, stop=True)
            inv_sb = small.tile([1, 1], F32, tag="inv_sb")
            nc.vector.reciprocal(inv_sb, p_tot)
            p_rep = p_misc.tile([128, 1], F32, tag="pmsc")
            nc.tensor.matmul(p_rep, lhsT=ones_row, rhs=inv_sb,
                             start=True, stop=True)
            inv_rep = small.tile([128, 1], F32, tag="inv_rep")
            nc.scalar.copy(inv_rep, p_rep)
            nc.vector.tensor_scalar_mul(w_b, w_b, inv_rep)

            # ---- attention_weights out: PE-transpose to [NCHUNK, 128] ----
            p_t = p_misc.tile([NCHUNK, 128], F32, tag="pmsc")
            nc.tensor.transpose(p_t, w_b, ident_sb)
            wT_sb = small.tile([NCHUNK, 128], F32, tag="wT_sb")
            nc.scalar.copy(wT_sb, p_t)
            nc.gpsimd.dma_start(out=d_ow.ap()[b].rearrange(
                "(c q) -> c q", c=NCHUNK), in_=wT_sb)

            # ---- context: ctx[b] = sum_t w[b, t] mem[b, t, :] ----
            pc = p_ctx.tile([1, EMB], F32)
            for h in range(2):
                mem_t = mempool.tile([128, 8, EMB], F32)
                nc.sync.dma_start(out=mem_t, in_=d_mem.ap()[b][
                    h * 1024:(h + 1) * 1024, :].rearrange(
                    "(j p) e -> p j e", p=128))
                for j in range(8):
                    c = h * 8 + j
                    nc.tensor.matmul(pc, lhsT=w_b[:, c:c + 1],
                                     rhs=mem_t[:, j, :],
                                     start=(c == 0), stop=(c == NCHUNK - 1))
            ctx_sb = small.tile([1, EMB], F32, tag="ctx_sb")
            nc.scalar.copy(ctx_sb, pc)
            nc.gpsimd.dma_start(out=d_octx.ap()[b].rearrange(
                "(o e) -> o e", o=1), in_=ctx_sb)

    nc.compile()
    return nc


def prep_core_inputs(attention_hidden_state, memory, processed_memory,
                     attention_weights_cat, mask, Wq, bq, conv_w, conv_b,
                     Wl, bl, v_w, v_b):
    """Host-side zero-flop prep + batch sharding -> per-core in_maps."""
    f = np.float32
    Wq = np.asarray(Wq, f)
    Wl = np.asarray(Wl, f)
    conv_w = np.asarray(conv_w, f)
    # Wcomb[(c, k), a] = sum_f conv_w[f, c, k] * Wl[a, f]  (weight folding)
    wcomb = np.ascontiguousarray(
        np.einsum("fck,af->cka", conv_w, Wl).reshape(KC, ATT).astype(f))
    brhs = np.ascontiguousarray(np.concatenate(
        [Wl.T, np.asarray(bq, f)[None, :], np.asarray(bl, f)[None, :]], axis=0))
    blhs = np.ascontiguousarray(np.concatenate(
        [np.tile(np.asarray(conv_b, f)[:, None], (1, BPC)),
         np.ones((2, BPC), f)], axis=0))
    wqT = np.ascontiguousarray(Wq.T)
    ones = np.ones((T,), f)
    ident = np.eye(128, dtype=f)
    vrow = np.ascontiguousarray(np.asarray(v_w, f)[0])

    awc = np.asarray(attention_weights_cat, f)
    awc_pad = np.zeros((B, 2, TP), f)
    awc_pad[:, :, PAD:PAD + T] = awc

    ahs = np.asarray(attention_hidden_state, f)
    mem = np.asarray(memory, f)
    pm = np.asarray(processed_memory, f)
    mask_u8 = np.asarray(mask).astype(np.uint8)

    in_maps = []
    for i in range(NCORES):
        s = slice(i * BPC, (i + 1) * BPC)
        in_maps.append({
            "ahsT": np.ascontiguousarray(ahs[s].T),
            "wqT": wqT,
            "blhs": blhs,
            "brhs": brhs,
            "awc": np.ascontiguousarray(awc_pad[s]),
            "wcomb": wcomb,
            "ones": ones,
            "vrow": vrow,
            "ident": ident,
            "pm": np.ascontiguousarray(pm[s]),
            "mem": np.ascontiguousarray(mem[s]),
            "maskb": np.ascontiguousarray(mask_u8[s]),
        })
    return in_maps


_CACHE = {}


def kernel(attention_hidden_state, memory, processed_memory,
           attention_weights_cat, mask, Wq, bq, conv_w, conv_b, Wl, bl,
           v_w, v_b, _trace=False):
    _install_ntff_shim()
    if "nc" not in _CACHE:
        _CACHE["nc"] = build_program()
    nc = _CACHE["nc"]
    in_maps = prep_core_inputs(
        attention_hidden_state, memory, processed_memory,
        attention_weights_cat, mask, Wq, bq, conv_w, conv_b, Wl, bl, v_w, v_b)
    res = run_bass_kernel_spmd(nc, in_maps, core_ids=list(range(NCORES)),
                               trace=_trace)
    _CACHE["last_exec_ns"] = res.exec_time_ns
    ctx = np.concatenate([res.results[i]["out_ctx"] for i in range(NCORES)], 0)
    wts = np.concatenate([res.results[i]["out_w"] for i in range(NCORES)], 0)
    return ctx, wts


# revision 8
# speedup vs baseline: 1.5573x; 1.5573x over previous
"""Trainium2 Bass kernel: location-sensitive attention (Tacotron2-style).

Contract: kernel(**inputs) takes the FULL numpy inputs and returns the full
(attention_context [B, EMB], attention_weights [B, T]) tuple.

Sharding: data-parallel over batch across 8 NeuronCores (8 batches/core),
weights replicated.  All tensor math runs on-device; host work is limited to
zero-flop layout prep (slicing, padding, transposes / constant folding of the
tiny weights).
"""

import contextlib
import ctypes
import sys
import types

import numpy as np
from ml_dtypes import bfloat16

import concourse.bacc as bacc
import concourse.bass as bass
import concourse.tile as tile
from concourse import mybir
from concourse.bass_utils import run_bass_kernel_spmd

F32 = mybir.dt.float32
BF16 = mybir.dt.bfloat16

B, T = 64, 2048
RNN, EMB, ATT, NF, KW = 1024, 512, 128, 32, 31
PAD = (KW - 1) // 2
NCORES = 8
BPC = B // NCORES          # batches per core
TP = T + 2 * PAD           # padded conv length
KC = 2 * KW                # im2col contraction dim (c, k)
NCHUNK = T // 128          # 16 t-chunks of 128


def _install_ntff_shim():
    """Provide antenv.axon_hooks (absent in some images) so trace=True can
    capture NTFF profiles through libaxon_pjrt.so.  No-op if unavailable."""
    if "antenv.axon_hooks" in sys.modules:
        return
    hook = None
    try:
        lib = ctypes.CDLL("/opt/axon/libaxon_pjrt.so")
        if hasattr(lib, "axon_start_nrt_profile"):
            lib.axon_start_nrt_profile.argtypes = [
                ctypes.POINTER(ctypes.c_int64),
                ctypes.c_size_t,
            ]
            lib.axon_start_nrt_profile.restype = ctypes.c_int64
            lib.axon_stop_nrt_profile.argtypes = [ctypes.c_char_p]
            lib.axon_stop_nrt_profile.restype = ctypes.c_int64

            @contextlib.contextmanager
            def _hook(output_dir, device_ids):
                import jax

                jax.devices()
                if device_ids:
                    ids = (ctypes.c_int64 * len(device_ids))(*device_ids)
                    rc = lib.axon_start_nrt_profile(ids, len(device_ids))
                else:
                    rc = lib.axon_start_nrt_profile(None, 0)
                if rc != 0:
                    raise RuntimeError(f"axon_start_nrt_profile rc={rc}")
                try:
                    yield
                finally:
                    n = lib.axon_stop_nrt_profile(str(output_dir).encode())
                    print(f"ntff profile: {n} file(s) in {output_dir}", file=sys.stderr)

            hook = _hook
    except OSError:
        pass
    mod = types.ModuleType("antenv.axon_hooks")
    mod.get_axon_ntff_profile_hook = lambda: hook
    mod.set_axon_ntff_profile_hook = lambda h: None
    sys.modules["antenv.axon_hooks"] = mod


def build_program():
    """Build and compile the per-core Bass program (SPMD, identical on all
    cores; only the input data differs)."""
    nc = bacc.Bacc("TRN2", target_bir_lowering=False, debug=False,
                   num_devices=NCORES)

    # ---- DRAM tensors (per-core shapes) ----
    d_ahsT = nc.dram_tensor("ahsT", [RNN, BPC], F32, kind="ExternalInput")
    d_wqT = nc.dram_tensor("wqT", [RNN, ATT], F32, kind="ExternalInput")
    d_blhs = nc.dram_tensor("blhs", [NF + 2, BPC], F32, kind="ExternalInput")
    d_brhs = nc.dram_tensor("brhs", [NF + 2, ATT], F32, kind="ExternalInput")
    d_awc = nc.dram_tensor("awc", [BPC, 2, TP], BF16, kind="ExternalInput")
    d_wcomb = nc.dram_tensor("wcomb", [KC, ATT], BF16, kind="ExternalInput")
    d_ones = nc.dram_tensor("ones", [T], F32, kind="ExternalInput")
    d_onesbf = nc.dram_tensor("onesbf", [T], BF16, kind="ExternalInput")
    d_v = nc.dram_tensor("vrow", [ATT], F32, kind="ExternalInput")
    d_ident = nc.dram_tensor("ident", [128, 128], F32, kind="ExternalInput")
    d_pm = nc.dram_tensor("pm", [BPC, T, ATT], F32, kind="ExternalInput")
    d_mem = nc.dram_tensor("mem", [BPC, T, EMB], BF16, kind="ExternalInput")
    d_mask = nc.dram_tensor("maskb", [BPC, T], mybir.dt.uint8,
                            kind="ExternalInput")
    d_octx = nc.dram_tensor("out_ctx", [BPC, EMB], F32, kind="ExternalOutput")
    d_ow = nc.dram_tensor("out_w", [BPC, T], F32, kind="ExternalOutput")

    with tile.TileContext(nc) as tc, contextlib.ExitStack() as ctx:
        consts = ctx.enter_context(tc.tile_pool(name="consts", bufs=1))
        xpool = ctx.enter_context(tc.tile_pool(name="xpool", bufs=3))
        rhspool = ctx.enter_context(tc.tile_pool(name="rhspool", bufs=3))
        pmpool = ctx.enter_context(tc.tile_pool(name="pmpool", bufs=2))
        mempool = ctx.enter_context(tc.tile_pool(name="mempool", bufs=3))
        ypool = ctx.enter_context(tc.tile_pool(name="ypool", bufs=3))
        small = ctx.enter_context(tc.tile_pool(name="small", bufs=3))
        p_e = ctx.enter_context(tc.tile_pool(name="p_e", bufs=3, space="PSUM"))
        p_ctx = ctx.enter_context(tc.tile_pool(name="p_ctx", bufs=2, space="PSUM"))
        p_misc = ctx.enter_context(tc.tile_pool(name="p_misc", bufs=3, space="PSUM"))

        # ---- resident constants ----
        wqT_sb = consts.tile([128, RNN // 128, ATT], F32)
        nc.sync.dma_start(out=wqT_sb, in_=d_wqT.ap().rearrange(
            "(a p) n -> p a n", p=128))
        ahsT_sb = consts.tile([128, RNN // 128, BPC], F32)
        nc.sync.dma_start(out=ahsT_sb, in_=d_ahsT.ap().rearrange(
            "(a p) b -> p a b", p=128))
        blhs_sb = consts.tile([NF + 2, BPC], F32)
        nc.sync.dma_start(out=blhs_sb, in_=d_blhs.ap())
        brhs_sb = consts.tile([NF + 2, ATT], F32)
        nc.sync.dma_start(out=brhs_sb, in_=d_brhs.ap())
        v_sb = consts.tile([128, ATT], F32)
        nc.sync.dma_start(out=v_sb, in_=bass.AP(
            tensor=d_v, offset=0, ap=[[0, 128], [1, ATT]]))
        ident_sb = consts.tile([128, 128], F32)
        nc.sync.dma_start(out=ident_sb, in_=d_ident.ap())
        ones_col = consts.tile([128, 1], F32)
        nc.sync.dma_start(out=ones_col, in_=d_ones.ap()[:128].rearrange(
            "(p o) -> p o", o=1))
        ones_row = consts.tile([1, 128], F32)
        nc.sync.dma_start(out=ones_row, in_=d_ones.ap()[:128].rearrange(
            "(o q) -> o q", o=1))
        pq_sb = consts.tile([BPC, ATT], F32)
        pq_bf = consts.tile([BPC, ATT], BF16)

        # ---- processed query + all bias terms, one accumulation group ----
        # pq[b, a] = sum_r ahs[b, r] Wq[a, r] + bq[a] + bl[a] + (conv_b @ Wl.T)[a]
        ppq = p_misc.tile([BPC, ATT], F32, tag="pmsc")
        for a in range(RNN // 128):
            nc.tensor.matmul(ppq, lhsT=ahsT_sb[:, a, :], rhs=wqT_sb[:, a, :],
                             start=(a == 0), stop=False)
        nc.tensor.matmul(ppq, lhsT=blhs_sb, rhs=brhs_sb, start=False, stop=True)
        nc.scalar.copy(pq_sb, ppq)
        nc.scalar.copy(pq_bf, ppq)

        for b in range(BPC):
            # ---- X im2col [63, T]: rows (c,k) = awc_pad[b, c, k:k+T], row 62 = ones
            # (PE stationary operand must sit at base partition 0/32/64)
            xt = xpool.tile([KC + 1, T], BF16)
            nc.sync.dma_start(
                out=xt[:KC, :],
                in_=bass.AP(tensor=d_awc, offset=b * 2 * TP,
                            ap=[[TP, 2], [1, KW], [1, T]]))
            nc.sync.dma_start(
                out=xt[KC:KC + 1, :],
                in_=d_onesbf.ap().rearrange("(o t) -> o t", o=1))

            # rhs for the fused conv+projection+bias matmul: [Wcomb; pq[b]]
            rhs_b = rhspool.tile([KC + 1, ATT], BF16)
            nc.gpsimd.dma_start(out=rhs_b[:KC, :], in_=d_wcomb.ap())
            nc.gpsimd.dma_start(out=rhs_b[KC:KC + 1, :], in_=pq_bf[b:b + 1, :])

            pm_t = pmpool.tile([128, NCHUNK, ATT], F32)
            nc.sync.dma_start(out=pm_t, in_=d_pm.ap()[b].rearrange(
                "(c p) a -> p c a", p=128))

            e_b = small.tile([128, NCHUNK], F32, tag="e_b")
            for c in range(NCHUNK):
                pe = p_e.tile([128, ATT], F32)
                nc.tensor.matmul(
                    pe, lhsT=xt[:, bass.ts(c, 128)],
                    rhs=rhs_b, start=True, stop=True)
                y = ypool.tile([128, ATT], F32, tag="y")
                nc.vector.tensor_add(y, pe, pm_t[:, c, :])
                th = ypool.tile([128, ATT], F32, tag="th")
                nc.scalar.activation(th, y, mybir.ActivationFunctionType.Tanh)
                prod = ypool.tile([128, ATT], F32, tag="prod")
                nc.vector.tensor_mul(prod, th, v_sb)
                # (+v_b dropped: softmax is shift-invariant, so it cancels)
                nc.vector.tensor_reduce(
                    out=e_b[:, c:c + 1], in_=prod,
                    axis=mybir.AxisListType.X, op=mybir.AluOpType.add)

            # ---- mask: energies += -1e30 * mask ----
            m8 = small.tile([128, NCHUNK], mybir.dt.uint8, tag="m8")
            nc.gpsimd.dma_start(out=m8, in_=d_mask.ap()[b].rearrange(
                "(c p) -> p c", p=128))
            mf = small.tile([128, NCHUNK], F32, tag="mf")
            nc.vector.tensor_copy(mf, m8)
            nc.vector.tensor_scalar_mul(mf, mf, -1e30)
            nc.vector.tensor_add(e_b, e_b, mf)

            # ---- softmax over T (energies bounded by sum|v|: no max pass) ----
            w_b = small.tile([128, NCHUNK], F32, tag="w_b")
            s_col = small.tile([128, 1], F32, tag="s_col")
            nc.scalar.activation(w_b, e_b, mybir.ActivationFunctionType.Exp,
                                 accum_out=s_col)
            p_tot = p_misc.tile([1, 1], F32, tag="pmsc")
            nc.tensor.matmul(p_tot, lhsT=s_col, rhs=ones_col,
                             start=True, stop=True)
            inv_sb = small.tile([1, 1], F32, tag="inv_sb")
            nc.vector.reciprocal(inv_sb, p_tot)
            p_rep = p_misc.tile([128, 1], F32, tag="pmsc")
            nc.tensor.matmul(p_rep, lhsT=ones_row, rhs=inv_sb,
                             start=True, stop=True)
            inv_rep = small.tile([128, 1], F32, tag="inv_rep")
            nc.scalar.copy(inv_rep, p_rep)
            nc.vector.tensor_scalar_mul(w_b, w_b, inv_rep)

            # ---- attention_weights out: PE-transpose to [NCHUNK, 128] ----
            p_t = p_misc.tile([NCHUNK, 128], F32, tag="pmsc")
            nc.tensor.transpose(p_t, w_b, ident_sb)
            wT_sb = small.tile([NCHUNK, 128], F32, tag="wT_sb")
            nc.scalar.copy(wT_sb, p_t)
            nc.gpsimd.dma_start(out=d_ow.ap()[b].rearrange(
                "(c q) -> c q", c=NCHUNK), in_=wT_sb)

            w_bf = small.tile([128, NCHUNK], BF16, tag="w_bf")
            nc.vector.tensor_copy(w_bf, w_b)

            # ---- context: ctx[b] = sum_t w[b, t] mem[b, t, :] ----
            pc = p_ctx.tile([1, EMB], F32)
            for h in range(2):
                mem_t = mempool.tile([128, 8, EMB], BF16)
                nc.sync.dma_start(out=mem_t, in_=d_mem.ap()[b][
                    h * 1024:(h + 1) * 1024, :].rearrange(
                    "(j p) e -> p j e", p=128))
                for j in range(8):
                    c = h * 8 + j
                    nc.tensor.matmul(pc, lhsT=w_bf[:, c:c + 1],
                                     rhs=mem_t[:, j, :],
                                     start=(c == 0), stop=(c == NCHUNK - 1))
            ctx_sb = small.tile([1, EMB], F32, tag="ctx_sb")
            nc.scalar.copy(ctx_sb, pc)
            nc.gpsimd.dma_start(out=d_octx.ap()[b].rearrange(
                "(o e) -> o e", o=1), in_=ctx_sb)

    nc.compile()
    return nc


def prep_core_inputs(attention_hidden_state, memory, processed_memory,
                     attention_weights_cat, mask, Wq, bq, conv_w, conv_b,
                     Wl, bl, v_w, v_b):
    """Host-side zero-flop prep + batch sharding -> per-core in_maps."""
    f = np.float32
    Wq = np.asarray(Wq, f)
    Wl = np.asarray(Wl, f)
    conv_w = np.asarray(conv_w, f)
    # Wcomb[(c, k), a] = sum_f conv_w[f, c, k] * Wl[a, f]  (weight folding)
    wcomb = np.ascontiguousarray(
        np.einsum("fck,af->cka", conv_w, Wl).reshape(KC, ATT).astype(bfloat16))
    brhs = np.ascontiguousarray(np.concatenate(
        [Wl.T, np.asarray(bq, f)[None, :], np.asarray(bl, f)[None, :]], axis=0))
    blhs = np.ascontiguousarray(np.concatenate(
        [np.tile(np.asarray(conv_b, f)[:, None], (1, BPC)),
         np.ones((2, BPC), f)], axis=0))
    wqT = np.ascontiguousarray(Wq.T)
    ones = np.ones((T,), f)
    ident = np.eye(128, dtype=f)
    vrow = np.ascontiguousarray(np.asarray(v_w, f)[0])

    awc = np.asarray(attention_weights_cat, f)
    awc_pad = np.zeros((B, 2, TP), bfloat16)
    awc_pad[:, :, PAD:PAD + T] = awc.astype(bfloat16)

    ahs = np.asarray(attention_hidden_state, f)
    mem = np.asarray(memory, f).astype(bfloat16)
    pm = np.asarray(processed_memory, f)
    mask_u8 = np.asarray(mask).astype(np.uint8)

    in_maps = []
    for i in range(NCORES):
        s = slice(i * BPC, (i + 1) * BPC)
        in_maps.append({
            "ahsT": np.ascontiguousarray(ahs[s].T),
            "wqT": wqT,
            "blhs": blhs,
            "brhs": brhs,
            "awc": np.ascontiguousarray(awc_pad[s]),
            "wcomb": wcomb,
            "ones": ones,
            "onesbf": ones.astype(bfloat16),
            "vrow": vrow,
            "ident": ident,
            "pm": np.ascontiguousarray(pm[s]),
            "mem": np.ascontiguousarray(mem[s]),
            "maskb": np.ascontiguousarray(mask_u8[s]),
        })
    return in_maps


_CACHE = {}


def kernel(attention_hidden_state, memory, processed_memory,
           attention_weights_cat, mask, Wq, bq, conv_w, conv_b, Wl, bl,
           v_w, v_b, _trace=False):
    _install_ntff_shim()
    if "nc" not in _CACHE:
        _CACHE["nc"] = build_program()
    nc = _CACHE["nc"]
    in_maps = prep_core_inputs(
        attention_hidden_state, memory, processed_memory,
        attention_weights_cat, mask, Wq, bq, conv_w, conv_b, Wl, bl, v_w, v_b)
    res = run_bass_kernel_spmd(nc, in_maps, core_ids=list(range(NCORES)),
                               trace=_trace)
    _CACHE["last_exec_ns"] = res.exec_time_ns
    ctx = np.concatenate([res.results[i]["out_ctx"] for i in range(NCORES)], 0)
    wts = np.concatenate([res.results[i]["out_w"] for i in range(NCORES)], 0)
    return ctx, wts


# revision 9
# speedup vs baseline: 1.7492x; 1.1232x over previous
"""Trainium2 Bass kernel: location-sensitive attention (Tacotron2-style).

Contract: kernel(**inputs) takes the FULL numpy inputs and returns the full
(attention_context [B, EMB], attention_weights [B, T]) tuple.

Sharding: data-parallel over batch across 8 NeuronCores (8 batches/core),
weights replicated.  All tensor math runs on-device; host work is limited to
zero-flop layout prep (slicing, padding, transposes / constant folding of the
tiny weights).
"""

import contextlib
import ctypes
import sys
import types

import numpy as np
from ml_dtypes import bfloat16

import concourse.bacc as bacc
import concourse.bass as bass
import concourse.tile as tile
from concourse import mybir
from concourse.bass_utils import run_bass_kernel_spmd

F32 = mybir.dt.float32
BF16 = mybir.dt.bfloat16

B, T = 64, 2048
RNN, EMB, ATT, NF, KW = 1024, 512, 128, 32, 31
PAD = (KW - 1) // 2
NCORES = 8
BPC = B // NCORES          # batches per core
TP = T + 2 * PAD           # padded conv length
KC = 2 * KW                # im2col contraction dim (c, k)
NCHUNK = T // 128          # 16 t-chunks of 128


def _install_ntff_shim():
    """Provide antenv.axon_hooks (absent in some images) so trace=True can
    capture NTFF profiles through libaxon_pjrt.so.  No-op if unavailable."""
    if "antenv.axon_hooks" in sys.modules:
        return
    hook = None
    try:
        lib = ctypes.CDLL("/opt/axon/libaxon_pjrt.so")
        if hasattr(lib, "axon_start_nrt_profile"):
            lib.axon_start_nrt_profile.argtypes = [
                ctypes.POINTER(ctypes.c_int64),
                ctypes.c_size_t,
            ]
            lib.axon_start_nrt_profile.restype = ctypes.c_int64
            lib.axon_stop_nrt_profile.argtypes = [ctypes.c_char_p]
            lib.axon_stop_nrt_profile.restype = ctypes.c_int64

            @contextlib.contextmanager
            def _hook(output_dir, device_ids):
                import jax

                jax.devices()
                if device_ids:
                    ids = (ctypes.c_int64 * len(device_ids))(*device_ids)
                    rc = lib.axon_start_nrt_profile(ids, len(device_ids))
                else:
                    rc = lib.axon_start_nrt_profile(None, 0)
                if rc != 0:
                    raise RuntimeError(f"axon_start_nrt_profile rc={rc}")
                try:
                    yield
                finally:
                    n = lib.axon_stop_nrt_profile(str(output_dir).encode())
                    print(f"ntff profile: {n} file(s) in {output_dir}", file=sys.stderr)

            hook = _hook
    except OSError:
        pass
    mod = types.ModuleType("antenv.axon_hooks")
    mod.get_axon_ntff_profile_hook = lambda: hook
    mod.set_axon_ntff_profile_hook = lambda h: None
    sys.modules["antenv.axon_hooks"] = mod


def build_program():
    """Build and compile the per-core Bass program (SPMD, identical on all
    cores; only the input data differs)."""
    nc = bacc.Bacc("TRN2", target_bir_lowering=False, debug=False,
                   num_devices=NCORES)

    # ---- DRAM tensors (per-core shapes) ----
    d_ahsT = nc.dram_tensor("ahsT", [RNN, BPC], F32, kind="ExternalInput")
    d_wqT = nc.dram_tensor("wqT", [RNN, ATT], F32, kind="ExternalInput")
    d_blhs = nc.dram_tensor("blhs", [NF + 2, BPC], F32, kind="ExternalInput")
    d_brhs = nc.dram_tensor("brhs", [NF + 2, ATT], F32, kind="ExternalInput")
    d_awc = nc.dram_tensor("awc", [BPC, 2, TP], BF16, kind="ExternalInput")
    d_wcomb = nc.dram_tensor("wcomb", [KC, ATT], BF16, kind="ExternalInput")
    d_ones = nc.dram_tensor("ones", [T], F32, kind="ExternalInput")
    d_onesbf = nc.dram_tensor("onesbf", [T], BF16, kind="ExternalInput")
    d_v = nc.dram_tensor("vrow", [ATT], F32, kind="ExternalInput")
    d_ident = nc.dram_tensor("ident", [128, 128], F32, kind="ExternalInput")
    d_identbf = nc.dram_tensor("identbf", [128, 128], BF16, kind="ExternalInput")
    d_pm = nc.dram_tensor("pm", [BPC, T, ATT], BF16, kind="ExternalInput")
    d_mem = nc.dram_tensor("mem", [BPC, T, EMB], BF16, kind="ExternalInput")
    d_mask = nc.dram_tensor("maskb", [BPC, T], mybir.dt.uint8,
                            kind="ExternalInput")
    d_octx = nc.dram_tensor("out_ctx", [BPC, EMB], F32, kind="ExternalOutput")
    d_ow = nc.dram_tensor("out_w", [BPC, T], F32, kind="ExternalOutput")

    with tile.TileContext(nc) as tc, contextlib.ExitStack() as ctx:
        consts = ctx.enter_context(tc.tile_pool(name="consts", bufs=1))
        xpool = ctx.enter_context(tc.tile_pool(name="xpool", bufs=3))
        rhspool = ctx.enter_context(tc.tile_pool(name="rhspool", bufs=3))
        pmpool = ctx.enter_context(tc.tile_pool(name="pmpool", bufs=2))
        mempool = ctx.enter_context(tc.tile_pool(name="mempool", bufs=3))
        ypool = ctx.enter_context(tc.tile_pool(name="ypool", bufs=3))
        small = ctx.enter_context(tc.tile_pool(name="small", bufs=3))
        p_e = ctx.enter_context(tc.tile_pool(name="p_e", bufs=3, space="PSUM"))
        p_ctx = ctx.enter_context(tc.tile_pool(name="p_ctx", bufs=2, space="PSUM"))
        p_misc = ctx.enter_context(tc.tile_pool(name="p_misc", bufs=3, space="PSUM"))

        # ---- resident constants ----
        wqT_sb = consts.tile([128, RNN // 128, ATT], F32)
        nc.sync.dma_start(out=wqT_sb, in_=d_wqT.ap().rearrange(
            "(a p) n -> p a n", p=128))
        ahsT_sb = consts.tile([128, RNN // 128, BPC], F32)
        nc.sync.dma_start(out=ahsT_sb, in_=d_ahsT.ap().rearrange(
            "(a p) b -> p a b", p=128))
        blhs_sb = consts.tile([NF + 2, BPC], F32)
        nc.sync.dma_start(out=blhs_sb, in_=d_blhs.ap())
        brhs_sb = consts.tile([NF + 2, ATT], F32)
        nc.sync.dma_start(out=brhs_sb, in_=d_brhs.ap())
        v_sb = consts.tile([128, 4, ATT], F32)
        nc.sync.dma_start(out=v_sb, in_=bass.AP(
            tensor=d_v, offset=0, ap=[[0, 128], [0, 4], [1, ATT]]))
        ident_sb = consts.tile([128, 128], F32)
        nc.sync.dma_start(out=ident_sb, in_=d_ident.ap())
        identbf_sb = consts.tile([128, 128], BF16)
        nc.sync.dma_start(out=identbf_sb, in_=d_identbf.ap())
        ones_col = consts.tile([128, 1], F32)
        nc.sync.dma_start(out=ones_col, in_=d_ones.ap()[:128].rearrange(
            "(p o) -> p o", o=1))
        ones_row = consts.tile([1, 128], F32)
        nc.sync.dma_start(out=ones_row, in_=d_ones.ap()[:128].rearrange(
            "(o q) -> o q", o=1))
        pq_sb = consts.tile([BPC, ATT], F32)
        pq_bf = consts.tile([BPC, ATT], BF16)

        # ---- processed query + all bias terms, one accumulation group ----
        # pq[b, a] = sum_r ahs[b, r] Wq[a, r] + bq[a] + bl[a] + (conv_b @ Wl.T)[a]
        ppq = p_misc.tile([BPC, ATT], F32, tag="pmsc")
        for a in range(RNN // 128):
            nc.tensor.matmul(ppq, lhsT=ahsT_sb[:, a, :], rhs=wqT_sb[:, a, :],
                             start=(a == 0), stop=False)
        nc.tensor.matmul(ppq, lhsT=blhs_sb, rhs=brhs_sb, start=False, stop=True)
        nc.scalar.copy(pq_sb, ppq)
        nc.scalar.copy(pq_bf, ppq)

        for b in range(BPC):
            # ---- X im2col [63, T]: rows (c,k) = awc_pad[b, c, k:k+T], row 62 = ones
            # (PE stationary operand must sit at base partition 0/32/64)
            xt = xpool.tile([KC + 1, T], BF16)
            nc.gpsimd.dma_start(
                out=xt[:KC, :],
                in_=bass.AP(tensor=d_awc, offset=b * 2 * TP,
                            ap=[[TP, 2], [1, KW], [1, T]]))
            nc.gpsimd.dma_start(
                out=xt[KC:KC + 1, :],
                in_=d_onesbf.ap().rearrange("(o t) -> o t", o=1))

            # rhs for the fused conv+projection+bias matmul: [Wcomb; pq[b]]
            rhs_b = rhspool.tile([KC + 1, ATT], BF16)
            nc.gpsimd.dma_start(out=rhs_b[:KC, :], in_=d_wcomb.ap())
            nc.gpsimd.dma_start(out=rhs_b[KC:KC + 1, :], in_=pq_bf[b:b + 1, :])

            pm_t = pmpool.tile([128, NCHUNK, ATT], BF16)
            nc.gpsimd.dma_start(out=pm_t, in_=d_pm.ap()[b].rearrange(
                "(c p) a -> p c a", p=128))

            e_b = small.tile([128, NCHUNK], F32, tag="e_b")
            for q in range(NCHUNK // 4):
                pe4 = p_e.tile([128, 4, ATT], F32)
                for j in range(4):
                    c = q * 4 + j
                    nc.tensor.matmul(
                        pe4[:, j, :], lhsT=xt[:, bass.ts(c, 128)],
                        rhs=rhs_b, start=True, stop=False)
                    # += pm via identity matmul (I.T @ pm = pm)
                    nc.tensor.matmul(
                        pe4[:, j, :], lhsT=identbf_sb,
                        rhs=pm_t[:, c, :], start=False, stop=True)
                th = ypool.tile([128, 4, ATT], F32, tag="th")
                nc.scalar.activation(th, pe4, mybir.ActivationFunctionType.Tanh)
                prod = ypool.tile([128, 4, ATT], F32, tag="prod")
                nc.vector.tensor_mul(prod, th, v_sb)
                # (+v_b dropped: softmax is shift-invariant, so it cancels)
                nc.vector.tensor_reduce(
                    out=e_b[:, q * 4:q * 4 + 4], in_=prod,
                    axis=mybir.AxisListType.X, op=mybir.AluOpType.add)

            # ---- mask: energies += -1e30 * mask ----
            m8 = small.tile([128, NCHUNK], mybir.dt.uint8, tag="m8")
            nc.gpsimd.dma_start(out=m8, in_=d_mask.ap()[b].rearrange(
                "(c p) -> p c", p=128))
            mf = small.tile([128, NCHUNK], F32, tag="mf")
            nc.vector.tensor_copy(mf, m8)
            nc.vector.tensor_scalar_mul(mf, mf, -1e30)
            nc.vector.tensor_add(e_b, e_b, mf)

            # ---- softmax over T (energies bounded by sum|v|: no max pass) ----
            w_b = small.tile([128, NCHUNK], F32, tag="w_b")
            s_col = small.tile([128, 1], F32, tag="s_col")
            nc.scalar.activation(w_b, e_b, mybir.ActivationFunctionType.Exp,
                                 accum_out=s_col)
            p_tot = p_misc.tile([1, 1], F32, tag="pmsc")
            nc.tensor.matmul(p_tot, lhsT=s_col, rhs=ones_col,
                             start=True, stop=True)
            inv_sb = small.tile([1, 1], F32, tag="inv_sb")
            nc.vector.reciprocal(inv_sb, p_tot)
            p_rep = p_misc.tile([128, 1], F32, tag="pmsc")
            nc.tensor.matmul(p_rep, lhsT=ones_row, rhs=inv_sb,
                             start=True, stop=True)
            inv_rep = small.tile([128, 1], F32, tag="inv_rep")
            nc.scalar.copy(inv_rep, p_rep)
            nc.vector.tensor_scalar_mul(w_b, w_b, inv_rep)

            # ---- attention_weights out: PE-transpose to [NCHUNK, 128] ----
            p_t = p_misc.tile([NCHUNK, 128], F32, tag="pmsc")
            nc.tensor.transpose(p_t, w_b, ident_sb)
            wT_sb = small.tile([NCHUNK, 128], F32, tag="wT_sb")
            nc.scalar.copy(wT_sb, p_t)
            nc.gpsimd.dma_start(out=d_ow.ap()[b].rearrange(
                "(c q) -> c q", c=NCHUNK), in_=wT_sb)

            w_bf = small.tile([128, NCHUNK], BF16, tag="w_bf")
            nc.vector.tensor_copy(w_bf, w_b)

            # ---- context: ctx[b] = sum_t w[b, t] mem[b, t, :] ----
            pc = p_ctx.tile([1, EMB], F32)
            for h in range(2):
                mem_t = mempool.tile([128, 8, EMB], BF16)
                nc.sync.dma_start(out=mem_t, in_=d_mem.ap()[b][
                    h * 1024:(h + 1) * 1024, :].rearrange(
                    "(j p) e -> p j e", p=128))
                for j in range(8):
                    c = h * 8 + j
                    nc.tensor.matmul(pc, lhsT=w_bf[:, c:c + 1],
                                     rhs=mem_t[:, j, :],
                                     start=(c == 0), stop=(c == NCHUNK - 1))
            ctx_sb = small.tile([1, EMB], F32, tag="ctx_sb")
            nc.scalar.copy(ctx_sb, pc)
            nc.gpsimd.dma_start(out=d_octx.ap()[b].rearrange(
                "(o e) -> o e", o=1), in_=ctx_sb)

    nc.compile()
    return nc


def prep_core_inputs(attention_hidden_state, memory, processed_memory,
                     attention_weights_cat, mask, Wq, bq, conv_w, conv_b,
                     Wl, bl, v_w, v_b):
    """Host-side zero-flop prep + batch sharding -> per-core in_maps."""
    f = np.float32
    Wq = np.asarray(Wq, f)
    Wl = np.asarray(Wl, f)
    conv_w = np.asarray(conv_w, f)
    # Wcomb[(c, k), a] = sum_f conv_w[f, c, k] * Wl[a, f]  (weight folding)
    wcomb = np.ascontiguousarray(
        np.einsum("fck,af->cka", conv_w, Wl).reshape(KC, ATT).astype(bfloat16))
    brhs = np.ascontiguousarray(np.concatenate(
        [Wl.T, np.asarray(bq, f)[None, :], np.asarray(bl, f)[None, :]], axis=0))
    blhs = np.ascontiguousarray(np.concatenate(
        [np.tile(np.asarray(conv_b, f)[:, None], (1, BPC)),
         np.ones((2, BPC), f)], axis=0))
    wqT = np.ascontiguousarray(Wq.T)
    ones = np.ones((T,), f)
    ident = np.eye(128, dtype=f)
    vrow = np.ascontiguousarray(np.asarray(v_w, f)[0])

    awc = np.asarray(attention_weights_cat, f)
    awc_pad = np.zeros((B, 2, TP), bfloat16)
    awc_pad[:, :, PAD:PAD + T] = awc.astype(bfloat16)

    ahs = np.asarray(attention_hidden_state, f)
    mem = np.asarray(memory, f).astype(bfloat16)
    pm = np.asarray(processed_memory, f).astype(bfloat16)
    mask_u8 = np.asarray(mask).astype(np.uint8)

    in_maps = []
    for i in range(NCORES):
        s = slice(i * BPC, (i + 1) * BPC)
        in_maps.append({
            "ahsT": np.ascontiguousarray(ahs[s].T),
            "wqT": wqT,
            "blhs": blhs,
            "brhs": brhs,
            "awc": np.ascontiguousarray(awc_pad[s]),
            "wcomb": wcomb,
            "ones": ones,
            "onesbf": ones.astype(bfloat16),
            "vrow": vrow,
            "ident": ident,
            "identbf": ident.astype(bfloat16),
            "pm": np.ascontiguousarray(pm[s]),
            "mem": np.ascontiguousarray(mem[s]),
            "maskb": np.ascontiguousarray(mask_u8[s]),
        })
    return in_maps


_CACHE = {}


def kernel(attention_hidden_state, memory, processed_memory,
           attention_weights_cat, mask, Wq, bq, conv_w, conv_b, Wl, bl,
           v_w, v_b, _trace=False):
    _install_ntff_shim()
    if "nc" not in _CACHE:
        _CACHE["nc"] = build_program()
    nc = _CACHE["nc"]
    in_maps = prep_core_inputs(
        attention_hidden_state, memory, processed_memory,
        attention_weights_cat, mask, Wq, bq, conv_w, conv_b, Wl, bl, v_w, v_b)
    res = run_bass_kernel_spmd(nc, in_maps, core_ids=list(range(NCORES)),
                               trace=_trace)
    _CACHE["last_exec_ns"] = res.exec_time_ns
    ctx = np.concatenate([res.results[i]["out_ctx"] for i in range(NCORES)], 0)
    wts = np.concatenate([res.results[i]["out_w"] for i in range(NCORES)], 0)
    return ctx, wts
